# revision 22
# baseline (speedup 1.0000x reference)
"""HardTripletLoss Trainium2 kernel.

Reference computation (B=256, C=1000, D=300):
  relations[b,c] = ||emb[b*C+c] - att[b*C+c] + 1e-6||_2          [B, C]
  hardest_positive[c] = max_b relations[b,c] * onehot(labels)[b,c]
  mx[c]              = max_b relations[b,c]
  hardest_negative[c] = min_b (relations[b,c] + mx[c]*onehot[b,c])
  loss = sum(relu(hp - hn + 1)) / (count(relu(...) > 1e-16) + 1e-16)

Sharding: data-parallel over B across 8 cores (32 b's per core, each a
contiguous 32000-row chunk of the (B*C, D) tensors). Each core computes
squared distances and 4 per-class partial reductions [C]:
  cmax = max_b rel_sq            pmax = max_b over positives of rel_sq
  umin = min_b over negatives    mmin = min_b over positives
(masking is exact: +/-1e30 select-style masks via min/max ALU ops, no
additive-cancellation error). Host all-reduces the [4, C] partials over
cores, takes sqrt (monotone, commutes with max/min), and finishes the
tiny [C]-sized tail: hn = min(umin, cmax_r + mmin), loss scalar.

Performance design (HW exec ~= DMA roofline):
- The kernel is HBM-bound; this core's fabric sustains ~195-200 GB/s.
  Inputs are pre-cast to fp16 ON THE HOST, halving device bytes to
  38.4 MB/core (loss err ~1e-5, tolerance 2e-2).
- All bulk loads go through SWDGE (gpsimd dma_start): its packets
  round-robin over all 16 DMA engines (22.5 GB/s each). HWDGE queues
  are pinned to engines 64-68 only and bottleneck at ~110 GB/s.
- Two b's per dma_start (250 descriptors each) halve the ~1 us/instr
  SWDGE descriptor-gen cost, shortening the issue ramp.
- Per pair: DVE subtract, one whole-pair ACT Square (8 small
  accum-ACT ops per b cost ~600 ns fixed each -- was a 227 us
  bottleneck), one DVE 3D row-sum, then 7 small contiguous DVE min/max
  ops fold the pair into running [125, 16] accumulators. Everything
  overlaps the DMA window; no strided post-pass tail.
- On-chip layout: partition p holds classes c = 8p + r, r in [0,8);
  each per-b line is 8 consecutive rows = 4.8 KB contiguous DRAM.
"""

import numpy as np

B, C, D = 256, 1000, 300
M = 8            # cores
BL = B // M      # 32 local anchors per core
P = 125          # partitions; partition p holds classes c = 8p + r
R = C // P       # 8 consecutive rows per partition
NP = BL // 2     # 16 b-pairs per core
BIG = 1.0e30
EPS_PD = 1e-6
MARGIN = 1.0

_STATE = {}


def _build():
    import concourse.tile as tile
    from concourse import bacc, mybir

    nc = bacc.Bacc("TRN2", target_bir_lowering=False, debug=False,
                   num_devices=M, num_swdge_queues=4)
    dt = mybir.dt.float32
    dt16 = mybir.dt.float16
    emb = nc.dram_tensor("emb", [BL * C, D], dt16, kind="ExternalInput").ap()
    att = nc.dram_tensor("att", [BL * C, D], dt16, kind="ExternalInput").ap()
    msk = nc.dram_tensor("msk", [P, BL * R], dt, kind="ExternalInput").ap()
    out = nc.dram_tensor("out", [P, 4 * R], dt, kind="ExternalOutput").ap()

    # row = b*1000 + p*8 + r; pair view: [bb, p, two, r, d]
    emb_v = emb.rearrange("(bb two p r) d -> bb p two r d", bb=NP, two=2, p=P, r=R)
    att_v = att.rearrange("(bb two p r) d -> bb p two r d", bb=NP, two=2, p=P, r=R)

    Alu = mybir.AluOpType
    Act = mybir.ActivationFunctionType
    Ax = mybir.AxisListType

    G = 4                 # pairs per masked-update group
    GW = G * 2 * R        # 64 columns per group
    NG = NP // G          # 4 groups

    with tile.TileContext(nc) as tc:
        with (
            tc.tile_pool(name="io", bufs=9) as io_pool,
            tc.tile_pool(name="tmp", bufs=3) as tmp_pool,
            tc.tile_pool(name="small", bufs=1) as small_pool,
        ):
            mask_t = small_pool.tile([P, BL * R], dt, tag="mask")
            nc.sync.dma_start(mask_t[:], msk[:])
            mask2_t = small_pool.tile([P, BL * R], dt, tag="mask2")
            nc.vector.tensor_scalar_mul(mask2_t[:], mask_t[:], -1.0)
            part_t = small_pool.tile([P, 4 * R], dt, tag="part")
            eps_t = small_pool.tile([P, 1], dt, tag="eps")
            nc.vector.memset(eps_t[:], EPS_PD)
            rel_t = small_pool.tile([P, BL * R], dt, tag="rel")
            # group-wide accumulators: col = (pair_in_group)*16 + two*8 + r
            cmax_t = small_pool.tile([P, GW], dt, tag="cmax")
            pmax_t = small_pool.tile([P, GW], dt, tag="pmax")
            umin_t = small_pool.tile([P, GW], dt, tag="umin")
            mmin_t = small_pool.tile([P, GW], dt, tag="mmin")
            nc.vector.memset(cmax_t[:], -BIG)
            nc.vector.memset(pmax_t[:], -BIG)
            nc.vector.memset(umin_t[:], BIG)
            nc.vector.memset(mmin_t[:], BIG)

            for bb in range(NP):
                e_t = io_pool.tile([P, 2, R, D], dt16, tag="e")
                nc.gpsimd.dma_start(e_t[:], emb_v[bb])
                a_t = io_pool.tile([P, 2, R, D], dt16, tag="a")
                nc.gpsimd.dma_start(a_t[:], att_v[bb])
                # in-place: a <- e - a (frees no extra tile), then the Square
                # writes back over e (same shape) -- no dif tiles at all, so
                # the io pool can run 9 pairs deep and the SWDGE issue stream
                # never stalls on buffer recycling
                nc.vector.tensor_sub(a_t[:], e_t[:], a_t[:])
                nc.scalar.activation(e_t[:], a_t[:], Act.Square,
                                     bias=eps_t[:], scale=1.0)
                s_t = e_t
                # f16 adds run 2 elem/cycle on DVE; the f32-accum reduce only
                # 1/cycle, so fold D 300->150->75 in f16 first
                f1_t = tmp_pool.tile([P, 2, R, 150], dt16, tag="f1")
                nc.vector.tensor_tensor(
                    f1_t[:], s_t[:, :, :, 0:150], s_t[:, :, :, 150:300],
                    op=Alu.add)
                f2_t = tmp_pool.tile([P, 2, R, 75], dt16, tag="f2")
                nc.vector.tensor_tensor(
                    f2_t[:], f1_t[:, :, :, 0:75], f1_t[:, :, :, 75:150],
                    op=Alu.add)
                nc.vector.tensor_reduce(
                    rel_t[:, bb * 2 * R:(bb + 1) * 2 * R], f2_t[:],
                    axis=Ax.X, op=Alu.add)

                if bb % G == G - 1:
                    g = bb // G
                    blk = rel_t[:, g * GW:(g + 1) * GW]
                    m_g = mask_t[:, g * GW:(g + 1) * GW]
                    m2_g = mask2_t[:, g * GW:(g + 1) * GW]
                    t_t = tmp_pool.tile([P, GW], dt, tag="t")
                    nc.vector.tensor_tensor(cmax_t[:], blk, cmax_t[:], op=Alu.max)
                    nc.vector.tensor_tensor(t_t[:], blk, m_g, op=Alu.min)
                    nc.vector.tensor_tensor(pmax_t[:], t_t[:], pmax_t[:], op=Alu.max)
                    nc.vector.tensor_tensor(t_t[:], blk, m_g, op=Alu.max)
                    nc.vector.tensor_tensor(umin_t[:], t_t[:], umin_t[:], op=Alu.min)
                    nc.vector.tensor_tensor(t_t[:], blk, m2_g, op=Alu.max)
                    nc.vector.tensor_tensor(mmin_t[:], t_t[:], mmin_t[:], op=Alu.min)

            # fold group-wide accumulators 64 -> 8 and assemble [P, 4R]
            for k, (acc, op) in enumerate([(cmax_t, Alu.max), (pmax_t, Alu.max),
                                           (umin_t, Alu.min), (mmin_t, Alu.min)]):
                h32 = tmp_pool.tile([P, 32], dt, tag="h32")
                nc.vector.tensor_tensor(h32[:], acc[:, 0:32], acc[:, 32:64], op=op)
                h16 = tmp_pool.tile([P, 16], dt, tag="h16")
                nc.vector.tensor_tensor(h16[:], h32[:, 0:16], h32[:, 16:32], op=op)
                nc.vector.tensor_tensor(
                    part_t[:, k * R:(k + 1) * R], h16[:, 0:8], h16[:, 8:16], op=op)
            nc.sync.dma_start(out[:], part_t[:])
    nc.compile()
    return nc


def _get_nc():
    if "nc" not in _STATE:
        _STATE["nc"] = _build()
    return _STATE["nc"]


def _make_masks(labels_np):
    """Per-core select masks msk[p, b*R+r] = +BIG if labels[b]==8p+r else -BIG."""
    masks = []
    c_of_pr = R * np.arange(P)[:, None] + np.arange(R)[None, :]     # [P, R]
    for m in range(M):
        lb = labels_np[m * BL:(m + 1) * BL].astype(np.int64)        # [BL]
        match = c_of_pr[:, None, :] == lb[None, :, None]            # [P, BL, R]
        mask = np.where(match, np.float32(BIG), np.float32(-BIG))
        masks.append(np.ascontiguousarray(mask.reshape(P, BL * R),
                                          dtype=np.float32))
    return masks


def _partials_from_out(o):
    """Device out [P, 4R] (col k*R+r, class c = R*p + r) -> [4, C] float64."""
    return np.transpose(o.astype(np.float64).reshape(P, 4, R),
                        (1, 0, 2)).reshape(4, C)


def _run_device(attributes, embeddings, labels_np, trace=False):
    from concourse.bass_utils import run_bass_kernel_spmd
    nc = _get_nc()
    masks = _make_masks(labels_np)
    attributes = np.ascontiguousarray(attributes.astype(np.float16, copy=False))
    embeddings = np.ascontiguousarray(embeddings.astype(np.float16, copy=False))
    in_maps = []
    for m in range(M):
        sl = slice(m * BL * C, (m + 1) * BL * C)
        in_maps.append({
            "emb": embeddings[sl],
            "att": attributes[sl],
            "msk": masks[m],
        })
    return run_bass_kernel_spmd(nc, in_maps, list(range(M)), trace=trace)


def _combine(results):
    """All-reduce the per-core [P, 4R] partials and finish the loss on host."""
    cmax = np.full(C, -np.inf)
    pmax = np.full(C, -np.inf)
    umin = np.full(C, np.inf)
    mmin = np.full(C, np.inf)
    for m in range(M):
        pk = _partials_from_out(results[m]["out"])
        cmax = np.maximum(cmax, pk[0])
        pmax = np.maximum(pmax, pk[1])
        umin = np.minimum(umin, pk[2])
        mmin = np.minimum(mmin, pk[3])
    # squared space -> distances (max/min commute with sqrt on [0, inf))
    mx = np.sqrt(np.maximum(cmax, 0.0))
    hp = np.sqrt(np.maximum(pmax, 0.0))       # -BIG (no positive) -> 0
    umin_r = np.sqrt(np.maximum(umin, 0.0))   # +BIG sentinel stays huge
    mmin_r = np.sqrt(np.maximum(mmin, 0.0))
    hn = np.minimum(umin_r, mx + mmin_r)
    triplet = np.maximum(hp - hn + MARGIN, 0.0)
    num_hard = np.sum(triplet > 1e-16)
    loss = np.sum(triplet) / (num_hard + 1e-16)
    return np.float32(loss)


def kernel(attributes, embeddings, labels):
    attributes = np.asarray(attributes)
    embeddings = np.asarray(embeddings)
    labels_np = np.asarray(labels)
    res = _run_device(attributes, embeddings, labels_np)
    return _combine(res.results)


# revision 23
# speedup vs baseline: 1.6512x; 1.6512x over previous
"""HardTripletLoss Trainium2 kernel.

Reference computation (B=256, C=1000, D=300):
  relations[b,c] = ||emb[b*C+c] - att[b*C+c] + 1e-6||_2          [B, C]
  hardest_positive[c] = max_b relations[b,c] * onehot(labels)[b,c]
  mx[c]              = max_b relations[b,c]
  hardest_negative[c] = min_b (relations[b,c] + mx[c]*onehot[b,c])
  loss = sum(relu(hp - hn + 1)) / (count(relu(...) > 1e-16) + 1e-16)

Sharding: data-parallel over B across 8 cores; core m owns the contiguous
32000-row slice [m*32000, (m+1)*32000) of the (B*C, D) tensors. The device
does the heavy part only: per-row squared distances rel_sq[row] =
sum_d (emb-att+eps)^2, shipped back as [128, 250] f32 (128 KB/core,
+0.3% traffic). The host reshapes to [B, C], takes sqrt, and runs the
reference's tiny [B,C] max/min/mask logic in numpy (~ms).

Performance design (HW exec ~= DMA roofline; measured cap ~200 GB/s/core):
- fp16 inputs (host pre-cast): halves device bytes to 38.4 MB/core.
  Final loss err ~3e-5 vs 2e-2 tolerance.
- All bulk loads on SWDGE (gpsimd): packets round-robin over all 16 DMA
  engines (22.5 GB/s each). HWDGE queues are pinned to engines 64-68
  only and bottleneck at ~110 GB/s.
- Rows-contiguous layout: partition p holds rows [p*250, (p+1)*250), so
  a 25-row chunk DMA is [128, 15 KB] -- 128 descriptors, 15 KB packets.
  Fewer descriptors per DMA matters because SWDGE issue lock-steps on a
  small completion-sem ring (~8 DMAs in flight): with 250-desc pair DMAs
  the issue stream stalled 10-14 us at a time; 20 chunk DMAs of 1.92 MB
  keep ~15 MB queued ahead of the engines.
- Per chunk: in-place DVE subtract, one ACT Square (bias=eps), two f16
  fold-adds (2 elem/cycle) + one f32 3D row-sum reduce on DVE. All
  compute engines run far below the DMA window.
"""

import numpy as np

B, C, D = 256, 1000, 300
M = 8              # cores
BL = B // M        # 32 local anchors per core
ROWS = BL * C      # 32000 rows per core
P = 128            # partitions; partition p holds rows [p*RPP, (p+1)*RPP)
RPP = ROWS // P    # 250 rows per partition
RPC = 25           # rows per chunk (per partition)
NCH = RPP // RPC   # 10 chunks
EPS_PD = 1e-6
MARGIN = 1.0

_STATE = {}


def _build():
    import concourse.tile as tile
    from concourse import bacc, mybir

    nc = bacc.Bacc("TRN2", target_bir_lowering=False, debug=False,
                   num_devices=M, num_swdge_queues=4)
    dt = mybir.dt.float32
    dt16 = mybir.dt.float16
    emb = nc.dram_tensor("emb", [ROWS, D], dt16, kind="ExternalInput").ap()
    att = nc.dram_tensor("att", [ROWS, D], dt16, kind="ExternalInput").ap()
    out = nc.dram_tensor("out", [P, RPP], dt, kind="ExternalOutput").ap()

    # row = p*250 + j*25 + w  ->  chunk view [j, p, w, d]
    emb_v = emb.rearrange("(p j w) d -> j p w d", p=P, j=NCH, w=RPC)
    att_v = att.rearrange("(p j w) d -> j p w d", p=P, j=NCH, w=RPC)

    Alu = mybir.AluOpType
    Act = mybir.ActivationFunctionType
    Ax = mybir.AxisListType

    with tile.TileContext(nc) as tc:
        with (
            tc.tile_pool(name="io", bufs=5) as io_pool,
            tc.tile_pool(name="tmp", bufs=3) as tmp_pool,
            tc.tile_pool(name="small", bufs=1) as small_pool,
        ):
            eps_t = small_pool.tile([P, 1], dt, tag="eps")
            nc.vector.memset(eps_t[:], EPS_PD)
            rel_t = small_pool.tile([P, RPP], dt, tag="rel")

            for j in range(NCH):
                e_t = io_pool.tile([P, RPC, D], dt16, tag="e")
                nc.gpsimd.dma_start(e_t[:], emb_v[j])
                a_t = io_pool.tile([P, RPC, D], dt16, tag="a")
                nc.gpsimd.dma_start(a_t[:], att_v[j])
                # in-place diff then Square back over e: no extra tiles, so
                # the io pool runs deep and DMA issue never waits on compute
                nc.vector.tensor_sub(a_t[:], e_t[:], a_t[:])
                nc.scalar.activation(e_t[:], a_t[:], Act.Square,
                                     bias=eps_t[:], scale=1.0)
                # f16 adds run 2 elem/cycle on DVE; the f32-accum reduce only
                # 1/cycle, so fold D 300->150->75 in f16 first
                f1_t = tmp_pool.tile([P, RPC, 150], dt16, tag="f1")
                nc.vector.tensor_tensor(
                    f1_t[:], e_t[:, :, 0:150], e_t[:, :, 150:300], op=Alu.add)
                f2_t = tmp_pool.tile([P, RPC, 75], dt16, tag="f2")
                nc.vector.tensor_tensor(
                    f2_t[:], f1_t[:, :, 0:75], f1_t[:, :, 75:150], op=Alu.add)
                nc.vector.tensor_reduce(
                    rel_t[:, j * RPC:(j + 1) * RPC], f2_t[:],
                    axis=Ax.X, op=Alu.add)

            nc.sync.dma_start(out[:], rel_t[:])
    nc.compile()
    return nc


def _get_nc():
    if "nc" not in _STATE:
        _STATE["nc"] = _build()
    return _STATE["nc"]


def _run_device(attributes, embeddings, labels_np, trace=False):
    from concourse.bass_utils import run_bass_kernel_spmd
    nc = _get_nc()
    attributes = np.ascontiguousarray(attributes.astype(np.float16, copy=False))
    embeddings = np.ascontiguousarray(embeddings.astype(np.float16, copy=False))
    in_maps = []
    for m in range(M):
        sl = slice(m * ROWS, (m + 1) * ROWS)
        in_maps.append({
            "emb": embeddings[sl],
            "att": attributes[sl],
        })
    return run_bass_kernel_spmd(nc, in_maps, list(range(M)), trace=trace)


def _combine(results, labels_np):
    """Assemble [B, C] relations from per-core row-sums; finish on host."""
    rel_sq = np.concatenate(
        [np.asarray(r["out"], dtype=np.float64).reshape(ROWS) for r in results]
    ).reshape(B, C)
    relations = np.sqrt(np.maximum(rel_sq, 0.0))
    mask_pos = np.zeros((B, C), dtype=np.float64)
    mask_pos[np.arange(B), labels_np.astype(np.int64)] = 1.0
    hp = (relations * mask_pos).max(axis=0)
    mx = relations.max(axis=0)
    hn = (relations + mx[None, :] * mask_pos).min(axis=0)
    triplet = np.maximum(hp - hn + MARGIN, 0.0)
    num_hard = np.sum(triplet > 1e-16)
    loss = np.sum(triplet) / (num_hard + 1e-16)
    return np.float32(loss)


def kernel(attributes, embeddings, labels):
    attributes = np.asarray(attributes)
    embeddings = np.asarray(embeddings)
    labels_np = np.asarray(labels)
    res = _run_device(attributes, embeddings, labels_np)
    return _combine(res.results, labels_np)


# revision 24
# speedup vs baseline: 1.8162x; 1.0999x over previous
"""HardTripletLoss Trainium2 kernel.

Reference computation (B=256, C=1000, D=300):
  relations[b,c] = ||emb[b*C+c] - att[b*C+c] + 1e-6||_2          [B, C]
  hardest_positive[c] = max_b relations[b,c] * onehot(labels)[b,c]
  mx[c]              = max_b relations[b,c]
  hardest_negative[c] = min_b (relations[b,c] + mx[c]*onehot[b,c])
  loss = sum(relu(hp - hn + 1)) / (count(relu(...) > 1e-16) + 1e-16)

Sharding: data-parallel over B across 8 cores; core m owns the contiguous
32000-row slice [m*32000, (m+1)*32000) of the (B*C, D) tensors. The device
does the heavy part only: per-row squared distances rel_sq[row] =
sum_d (emb-att+eps)^2, shipped back as [128, 250] f32 (128 KB/core,
+0.3% traffic). The host reshapes to [B, C], takes sqrt, and runs the
reference's tiny [B,C] max/min/mask logic in numpy (~ms).

Performance design (HW exec ~= DMA roofline; measured cap ~200 GB/s/core):
- fp16 inputs (host pre-cast): halves device bytes to 38.4 MB/core.
  Final loss err ~3e-5 vs 2e-2 tolerance.
- All bulk loads on SWDGE (gpsimd): packets round-robin over all 16 DMA
  engines (22.5 GB/s each). HWDGE queues are pinned to engines 64-68
  only and bottleneck at ~110 GB/s.
- Rows-contiguous layout: partition p holds rows [p*250, (p+1)*250), so
  a 25-row chunk DMA is [128, 15 KB] -- 128 descriptors, 15 KB packets.
  Fewer descriptors per DMA matters because SWDGE issue lock-steps on a
  small completion-sem ring (~8 DMAs in flight): with 250-desc pair DMAs
  the issue stream stalled 10-14 us at a time; 20 chunk DMAs of 1.92 MB
  keep ~15 MB queued ahead of the engines.
- Per chunk: in-place DVE subtract, one ACT Square (bias=eps), two f16
  fold-adds (2 elem/cycle) + one f32 3D row-sum reduce on DVE. All
  compute engines run far below the DMA window.
"""

import numpy as np

B, C, D = 256, 1000, 300
M = 8              # cores
BL = B // M        # 32 local anchors per core
ROWS = BL * C      # 32000 rows per core
P = 128            # partitions; partition p holds rows [p*RPP, (p+1)*RPP)
RPP = ROWS // P    # 250 rows per partition
RPC = 25           # rows per chunk (per partition)
NCH = RPP // RPC   # 10 chunks
EPS_PD = 1e-6
MARGIN = 1.0

_STATE = {}


def _build():
    import concourse.tile as tile
    from concourse import bacc, mybir

    nc = bacc.Bacc("TRN2", target_bir_lowering=False, debug=False,
                   num_devices=M, num_swdge_queues=4)
    dt = mybir.dt.float32
    dt16 = mybir.dt.float16
    emb = nc.dram_tensor("emb", [ROWS, D], dt16, kind="ExternalInput").ap()
    att = nc.dram_tensor("att", [ROWS, D], dt16, kind="ExternalInput").ap()
    out = nc.dram_tensor("out", [P, RPP], dt, kind="ExternalOutput").ap()

    # row = p*250 + j*25 + w  ->  chunk view [j, p, w, d]
    emb_v = emb.rearrange("(p j w) d -> j p w d", p=P, j=NCH, w=RPC)
    att_v = att.rearrange("(p j w) d -> j p w d", p=P, j=NCH, w=RPC)

    Alu = mybir.AluOpType
    Act = mybir.ActivationFunctionType
    Ax = mybir.AxisListType

    with tile.TileContext(nc) as tc:
        with (
            tc.tile_pool(name="io", bufs=5) as io_pool,
            tc.tile_pool(name="tmp", bufs=3) as tmp_pool,
            tc.tile_pool(name="small", bufs=1) as small_pool,
        ):
            eps_t = small_pool.tile([P, 1], dt, tag="eps")
            nc.vector.memset(eps_t[:], EPS_PD)
            rel_t = small_pool.tile([P, RPP], dt, tag="rel")

            def folds(s_t, j):
                # f16 adds run 2 elem/cycle on DVE; the f32-accum reduce only
                # 1/cycle, so fold D 300->150->75 in f16 first
                f1_t = tmp_pool.tile([P, RPC, 150], dt16, tag="f1")
                nc.vector.tensor_tensor(
                    f1_t[:], s_t[:, :, 0:150], s_t[:, :, 150:300], op=Alu.add)
                f2_t = tmp_pool.tile([P, RPC, 75], dt16, tag="f2")
                nc.vector.tensor_tensor(
                    f2_t[:], f1_t[:, :, 0:75], f1_t[:, :, 75:150], op=Alu.add)
                nc.vector.tensor_reduce(
                    rel_t[:, j * RPC:(j + 1) * RPC], f2_t[:],
                    axis=Ax.X, op=Alu.add)

            # software pipeline: chunk j's folds are emitted AFTER chunk
            # j+1's subtract. Engine streams execute in order, so without
            # the skew DVE sits idle inside every chunk waiting for ACT's
            # Square (f1 reads it) -- that serial chain gated DMA issue at
            # ~15 us/chunk.
            pend = None
            for j in range(NCH):
                e_t = io_pool.tile([P, RPC, D], dt16, tag="e")
                nc.gpsimd.dma_start(e_t[:], emb_v[j])
                a_t = io_pool.tile([P, RPC, D], dt16, tag="a")
                nc.gpsimd.dma_start(a_t[:], att_v[j])
                # in-place diff then Square back over e: no extra tiles, so
                # the io pool runs deep and DMA issue never waits on compute
                nc.vector.tensor_sub(a_t[:], e_t[:], a_t[:])
                nc.scalar.activation(e_t[:], a_t[:], Act.Square,
                                     bias=eps_t[:], scale=1.0)
                if pend is not None:
                    folds(*pend)
                pend = (e_t, j)
            folds(*pend)

            nc.sync.dma_start(out[:], rel_t[:])
    nc.compile()
    return nc


def _get_nc():
    if "nc" not in _STATE:
        _STATE["nc"] = _build()
    return _STATE["nc"]


def _run_device(attributes, embeddings, labels_np, trace=False):
    from concourse.bass_utils import run_bass_kernel_spmd
    nc = _get_nc()
    attributes = np.ascontiguousarray(attributes.astype(np.float16, copy=False))
    embeddings = np.ascontiguousarray(embeddings.astype(np.float16, copy=False))
    in_maps = []
    for m in range(M):
        sl = slice(m * ROWS, (m + 1) * ROWS)
        in_maps.append({
            "emb": embeddings[sl],
            "att": attributes[sl],
        })
    return run_bass_kernel_spmd(nc, in_maps, list(range(M)), trace=trace)


def _combine(results, labels_np):
    """Assemble [B, C] relations from per-core row-sums; finish on host."""
    rel_sq = np.concatenate(
        [np.asarray(r["out"], dtype=np.float64).reshape(ROWS) for r in results]
    ).reshape(B, C)
    relations = np.sqrt(np.maximum(rel_sq, 0.0))
    mask_pos = np.zeros((B, C), dtype=np.float64)
    mask_pos[np.arange(B), labels_np.astype(np.int64)] = 1.0
    hp = (relations * mask_pos).max(axis=0)
    mx = relations.max(axis=0)
    hn = (relations + mx[None, :] * mask_pos).min(axis=0)
    triplet = np.maximum(hp - hn + MARGIN, 0.0)
    num_hard = np.sum(triplet > 1e-16)
    loss = np.sum(triplet) / (num_hard + 1e-16)
    return np.float32(loss)


def kernel(attributes, embeddings, labels):
    attributes = np.asarray(attributes)
    embeddings = np.asarray(embeddings)
    labels_np = np.asarray(labels)
    res = _run_device(attributes, embeddings, labels_np)
    return _combine(res.results, labels_np)
